# revision 15
# baseline (speedup 1.0000x reference)
"""GAT attention head (gnn_message_passing) on 8 TRN2 NeuronCores.

v3 design (single gather, pure-block tiles, chunked AllGather):
  - Nodes partitioned across 8 cores (12500 each); per core, dest nodes are
    LPT-balanced into NB=100 blocks of 128 slots (NSLOT=12800).
  - Node phase (region-major, 4 regions of 25 tiles): seq = feat @ W on PE,
    f1 = seq@a_l via fused DVE dot, bf16 seq rows staged per region and
    DMA'd to the AllGather input; after each region completes, a per-region
    AllGather replicates that slot-range of all cores into a region table
    (25600 rows < 32767 so gather indices fit int16).  No table copies.
  - f1 per dest slot is transposed once on PE, round-tripped to a single
    [1, NSLOT] partition-0 row via one SBUF->SBUF DMA, and
    partition-broadcast into f1rows [128, NSLOT] (bf16).
  - Edge phase: edges grouped by (dest block, source region), each
    (block, region) run padded to a multiple of 128 so every 128-edge tile
    is pure (single block, single region).  One dma_gather per
    (super, region) fetches 256B bf16 seq rows.  Per tile:
      F2 (f2 per edge) via fused DVE dot of G with a_r,
      f1e via fused DVE one-hot dot against f1rows (is_equal x mult, accum),
      w = exp(0.6t+0.4|t|+b) as cheap column ops,
      wt = (iota==rowrel)*w in ONE fused DVE op, and PE matmuls
      wt.T@[G] into psum[:, :128] and wt.T@[1] into psum[:, 128:129]
      accumulate numerator and softmax denominator per dest block.
  - Softmax max-subtraction is skipped (logits are O(1); exp safe in f32).

Host side does only index manipulation (partitioning, padding, permutation)
and parameter replication; all floating-point compute on feature data runs
on device.
"""

import math
import sys

import numpy as np

for _p in ("/opt/trn_rl_repo",):
    if _p not in sys.path:
        sys.path.insert(0, _p)

import concourse.bacc as bacc
import concourse.bass as bass
import concourse.mybir as mybir
import concourse.tile as tile
from concourse.bass_utils import run_bass_kernel_spmd

F32 = mybir.dt.float32
BF16 = mybir.dt.bfloat16
I16 = mybir.dt.int16
U8 = mybir.dt.uint8
AF = mybir.ActivationFunctionType
ALU = mybir.AluOpType


class _Cfg:
    def __init__(self, N, E, IN, OUT, C, sup_blocks=8, regions=4):
        assert N % C == 0
        self.N, self.E, self.IN, self.OUT, self.C = N, E, IN, OUT, C
        self.KI = IN // 128
        assert IN == self.KI * 128
        assert OUT == 128, "builder assumes OUT==128"
        self.NPC = N // C                     # nodes per core
        self.REG = regions
        # nodes are partitioned into REG quarters by original id; each
        # quarter gets ceil(QN/128)+1 blocks (one slack block for balance)
        self.QN = math.ceil(self.NPC / regions)   # nodes per quarter
        per_reg = math.ceil(self.QN / 128) + 1
        self.NB = per_reg * regions           # 104 for NPC=12500, REG=4
        self.BPQ = per_reg                    # blocks per quarter
        self.NSLOT = self.NB * 128
        self.RSLOT = self.NSLOT // regions    # slots per region per core
        self.RTB = self.RSLOT // 128          # node tiles per region
        self.RROWS = C * self.RSLOT           # region table rows
        assert self.RROWS <= 32767, "dma_gather int16 index range"
        self.SUP = sup_blocks
        self.supers = []
        b = 0
        while b < self.NB:
            nbl = min(sup_blocks, self.NB - b)
            self.supers.append((b, nbl))
            b += nbl
        self.meta = None


def _prep_host(cfg, feat, W, a_l, b_l, a_r, b_r, bias, row, col):
    C, NPC, NB, NSLOT = cfg.C, cfg.NPC, cfg.NB, cfg.NSLOT
    N, IN, OUT, REG, RSLOT = cfg.N, cfg.IN, cfg.OUT, cfg.REG, cfg.RSLOT

    row = row.astype(np.int64)
    col = col.astype(np.int64)
    core = row // NPC
    QN, BPQ = cfg.QN, cfg.BPQ

    # node -> original-id quarter (this IS the source region, independent of
    # the block assignment below)
    oq = np.minimum((np.arange(N) % NPC) // QN, REG - 1)

    # per-dest in-edge counts split by source region
    vreg = np.zeros((N, REG), np.int64)
    np.add.at(vreg, (row, oq[col]), 1)
    deg = vreg.sum(axis=1)

    # --- region-aware LPT: per (core, quarter), balance per-region loads
    # across the quarter's BPQ blocks (capacity 128 dests each) ------------
    newlocal = np.empty(N, np.int64)
    for c in range(C):
        for q in range(REG):
            n0 = c * NPC + q * QN
            n1 = min(c * NPC + NPC, n0 + QN)
            ids = np.arange(n0, n1)
            order = ids[np.argsort(-deg[n0:n1], kind="stable")]
            loads = np.zeros((BPQ, REG), np.int64)
            counts = np.zeros(BPQ, np.int64)
            vq = vreg[order]
            for i, dest in enumerate(order):
                cand = (loads + vq[i]).max(axis=1)
                cand[counts >= 128] = 1 << 50
                b = int(np.argmin(cand))
                newlocal[dest] = (q * BPQ + b) * 128 + counts[b]
                counts[b] += 1
                loads[b] += vq[i]

    # --- per-edge derived ids --------------------------------------------
    snl = newlocal[col]                       # source slot within its core
    ereg = oq[col]                            # source region (= slot quarter)
    erow = (col // NPC) * RSLOT + (snl - ereg * RSLOT)  # region-local row
    edslot = newlocal[row]                    # dest slot
    eblk = edslot // 128
    epos = (edslot % 128).astype(np.float32)

    # counts per (core, block, region); SPMD => pad to max over cores
    cnts = np.zeros((C, NB, REG), np.int64)
    np.add.at(cnts, (core, eblk, ereg), 1)
    runmax = cnts.max(axis=0)                 # [NB, REG]
    tiles_br = (runmax + 127) // 128          # tiles per (block, region)

    # --- tile layout ------------------------------------------------------
    # G columns are laid out region-major within each super (matching the
    # gather stream); processing is block-major (PSUM locality).
    meta = {"supers": []}
    gtile = 0
    t0_br = np.full((NB, REG), -1, np.int64)  # global first tile of run
    for (b0, nbl) in cfg.supers:
        sup = {"b0": b0, "nb": nbl, "gt0": gtile, "g_calls": [], "blocks": []}
        scol = 0
        for r in range(REG):
            nt_r = int(tiles_br[b0:b0 + nbl, r].sum())
            if nt_r:
                sup["g_calls"].append(
                    {"region": r, "tile0": scol, "ntiles": nt_r})
            for bi in range(nbl):
                t0_br[b0 + bi, r] = gtile + scol
                scol += int(tiles_br[b0 + bi, r])
        sup["ntiles"] = scol
        for bi in range(nbl):
            b = b0 + bi
            runs = [(int(t0_br[b, r]), int(tiles_br[b, r]))
                    for r in range(REG) if tiles_br[b, r] > 0]
            sup["blocks"].append({"b": b, "runs": runs})
        gtile += scol
        meta["supers"].append(sup)
    NTILES = gtile
    meta["NTILES"] = NTILES

    # --- per-core index arrays -------------------------------------------
    idxg = np.zeros((C, 128, NTILES * 8), np.int16)
    rowrel = np.full((C, 128, NTILES), -1.0, np.float32)

    # slot of each edge within its (core, block, region) run
    okey = (core * NB + eblk) * REG + ereg
    oorder = np.argsort(okey, kind="stable")
    ks = okey[oorder]
    starts = np.searchsorted(ks, np.arange(C * NB * REG))
    slot_in_run = np.empty(cfg.E, np.int64)
    slot_in_run[oorder] = np.arange(cfg.E) - starts[ks]

    gt0_e = t0_br[eblk, ereg]                 # first global tile of the run
    ek = gt0_e * 128 + slot_in_run            # global slot id
    etile = ek // 128
    epart = ek % 128

    for c in range(C):
        m = core == c
        rowrel[c, epart[m], etile[m]] = epos[m]
        # wrapped idx layout: slot k -> [k%16, (k//16)], replicated to the
        # 8 groups of 16 partitions
        kk = ek[m]
        idxg[c, kk % 16, (kk // 16)] = erow[m].astype(np.int16)
    for g in range(1, 8):
        idxg[:, g * 16:(g + 1) * 16, :] = idxg[:, 0:16, :]

    # --- parameters -------------------------------------------------------
    inv = np.empty((C, NSLOT), np.int64)
    have = np.zeros((C, NSLOT), bool)
    for c in range(C):
        nl = newlocal[c * NPC:(c + 1) * NPC]
        inv[c, nl] = np.arange(NPC)
        have[c, nl] = True
    featT = np.zeros((C, IN, NSLOT), np.float32)
    for c in range(C):
        idx = inv[c][have[c]]
        featT[c][:, have[c]] = feat[c * NPC + idx].T
    wks = [np.ascontiguousarray(W[k * 128:(k + 1) * 128]).astype(np.float32)
           for k in range(cfg.KI)]
    alb = np.tile(np.asarray(a_l, np.float32)[None, :], (128, 1))
    arb = np.tile(np.asarray(a_r, np.float32).astype(np.float32)[None, :],
                  (128, 1)).astype(np.float32)
    arbh = arb.astype(np.float32)
    biasb = np.tile(np.asarray(bias, np.float32)[None, :], (128, 1))
    bsum = float(np.asarray(b_l, np.float64) + np.asarray(b_r, np.float64))
    bs_col = np.full((128, 1), bsum, np.float32)
    iota = np.tile(np.arange(128, dtype=np.float32)[None, :], (128, 1))
    ident = np.eye(128, dtype=np.float32)

    import ml_dtypes
    iotab = iota.astype(ml_dtypes.bfloat16)
    arb_bf = arbh.astype(ml_dtypes.bfloat16)

    in_maps = []
    for c in range(C):
        m = {
            "featT": featT[c], "alb": alb, "arbh": arb_bf, "biasb": biasb,
            "bs": bs_col, "iotab": iotab, "ident": ident,
            "idxg": idxg[c], "rowrel": rowrel[c],
        }
        for k in range(cfg.KI):
            m[f"wk{k}"] = wks[k]
        in_maps.append(m)

    cfg.meta = meta

    def assemble(outs):
        full = np.empty((N, OUT), np.float32)
        for c in range(C):
            o = outs[c]["out"]
            nlc = newlocal[c * NPC:(c + 1) * NPC]
            full[c * NPC:(c + 1) * NPC] = o[nlc]
        return full

    return in_maps, assemble


def _build_program(cfg):
    C, IN, OUT = cfg.C, cfg.IN, cfg.OUT
    NB, NSLOT, KI, REG = cfg.NB, cfg.NSLOT, cfg.KI, cfg.REG
    RSLOT, RTB, RROWS = cfg.RSLOT, cfg.RTB, cfg.RROWS
    meta = cfg.meta
    NTILES = meta["NTILES"]
    NTMAX = max(s["ntiles"] for s in meta["supers"])
    FTC = RTB // 2 if RTB % 2 == 0 else RTB     # node tiles per featT chunk
    FCH = FTC * 128                             # featT load chunk columns

    nc = bacc.Bacc(None)
    featT = nc.declare_dram_parameter("featT", [IN, NSLOT], F32, isOutput=False)
    wk = [nc.declare_dram_parameter(f"wk{k}", [128, OUT], F32, isOutput=False)
          for k in range(KI)]
    alb = nc.declare_dram_parameter("alb", [128, OUT], F32, isOutput=False)
    arbh = nc.declare_dram_parameter("arbh", [128, OUT], BF16, isOutput=False)
    biasb = nc.declare_dram_parameter("biasb", [128, OUT], F32, isOutput=False)
    bs = nc.declare_dram_parameter("bs", [128, 1], F32, isOutput=False)
    iotab = nc.declare_dram_parameter("iotab", [128, 128], BF16, isOutput=False)
    ident = nc.declare_dram_parameter("ident", [128, 128], F32, isOutput=False)
    idxg = nc.declare_dram_parameter("idxg", [128, NTILES * 8], I16,
                                     isOutput=False)
    rowrel = nc.declare_dram_parameter("rowrel", [128, NTILES], F32,
                                       isOutput=False)
    outp = nc.declare_dram_parameter("out", [NSLOT, OUT], F32, isOutput=True)

    with tile.TileContext(nc) as tc:
        with (
            tc.tile_pool(name="dram", bufs=1, space="DRAM") as dram,
            tc.tile_pool(name="consts", bufs=1) as cp,
            tc.tile_pool(name="nfeat", bufs=2) as nfp,
            tc.tile_pool(name="naug", bufs=2) as nap,
            tc.tile_pool(name="nscr", bufs=2) as nsp,
            tc.tile_pool(name="npsum", bufs=2, space="PSUM") as npp,
            tc.tile_pool(name="eidx", bufs=2) as eip,
            tc.tile_pool(name="egath", bufs=2) as egp,
            tc.tile_pool(name="ecol", bufs=2) as ecp,
            tc.tile_pool(name="escr", bufs=2) as esc,
            tc.tile_pool(name="ewt", bufs=4) as ewp,
            tc.tile_pool(name="epsum", bufs=2, space="PSUM") as epp,
            tc.tile_pool(name="eout", bufs=3) as eop,
        ):
            agin = [dram.tile([RSLOT, OUT], BF16, name=f"agin{r}")
                    for r in range(REG)]
            tabr = [dram.tile([RROWS, OUT], BF16, name=f"tabr{r}",
                              addr_space="Shared") for r in range(REG)]
            f1dram = dram.tile([128, 128], BF16)

            # ---- constants ----
            wk_sb = []
            for k in range(KI):
                w_t = cp.tile([128, OUT], F32, name=f"wksb{k}")
                nc.sync.dma_start(w_t[:], wk[k][:])
                wk_sb.append(w_t)
            alb_sb = cp.tile([128, OUT], F32)
            nc.sync.dma_start(alb_sb[:], alb[:])
            arb_sb = cp.tile([128, OUT], BF16)
            nc.sync.dma_start(arb_sb[:], arbh[:])
            biasb_sb = cp.tile([128, OUT], F32)
            nc.sync.dma_start(biasb_sb[:], biasb[:])
            bs_sb = cp.tile([128, 1], F32)
            nc.sync.dma_start(bs_sb[:], bs[:])
            iota_sb = cp.tile([128, 128], BF16)
            nc.sync.dma_start(iota_sb[:], iotab[:])
            ident_sb = cp.tile([128, 128], F32)
            nc.sync.dma_start(ident_sb[:], ident[:])
            ones_sb = cp.tile([128, 1], BF16)
            nc.vector.memset(ones_sb[:], 1.0)
            f1acc = cp.tile([128, 128], F32)
            nc.vector.memset(f1acc[:], 0.0)
            f1rows = cp.tile([128, NSLOT], BF16)

            # ---- node phase (region-major) ----
            for r in range(REG):
                fts = {}
                for k in range(KI):
                    for h in range(RTB // FTC):
                        ft = nfp.tile([128, FCH], F32, name=f"ft{k}{h}")
                        c0 = r * RSLOT + h * FCH
                        nc.sync.dma_start(
                            ft[:], featT[k * 128:(k + 1) * 128, c0:c0 + FCH])
                        fts[(k, h)] = ft
                aug = nap.tile([128, RSLOT], BF16, name="aug")
                for ntl in range(RTB):
                    nt = r * RTB + ntl
                    ps = npp.tile([128, OUT], F32)
                    for k in range(KI):
                        h, off = divmod(ntl * 128, FCH)
                        ft = fts[(k, h)]
                        nc.tensor.matmul(ps[:], lhsT=ft[:, off:off + 128],
                                         rhs=wk_sb[k][:],
                                         start=(k == 0), stop=(k == KI - 1))
                    nc.vector.tensor_copy(aug[:, ntl * 128:(ntl + 1) * 128],
                                          ps[:])
                    scr1 = nsp.tile([128, OUT], F32, name="scr1")
                    nc.vector.scalar_tensor_tensor(
                        out=scr1[:], in0=ps[:], scalar=1.0, in1=alb_sb[:],
                        op0=ALU.mult, op1=ALU.mult,
                        accum_out=f1acc[:, nt:nt + 1])
                nc.sync.dma_start(
                    agin[r][:, :].rearrange("(t p) o -> p t o", p=128),
                    aug[:].rearrange("p (t o) -> p t o", o=OUT))
                # per-region AllGather into the region table
                nc.gpsimd.collective_compute(
                    "AllGather", ALU.bypass,
                    replica_groups=[list(range(C))],
                    ins=[agin[r].opt()],
                    outs=[tabr[r].opt()],
                )

            # ---- f1rows: f1acc [128(pos), NB] -> broadcast rows ----
            f1b = nsp.tile([128, 128], F32, name="f1b")
            nc.vector.tensor_scalar(out=f1b[:], in0=f1acc[:],
                                    scalar1=bs_sb[:], scalar2=None,
                                    op0=ALU.add)
            f1ps = npp.tile([128, 128], F32)
            nc.tensor.transpose(f1ps[:], f1b[:], ident_sb[:])
            f1sbT = nsp.tile([128, 128], BF16, name="f1sbT")
            nc.vector.tensor_copy(f1sbT[:], f1ps[:])
            nc.sync.dma_start(f1dram[:], f1sbT[:])
            f1stage = cp.tile([1, NSLOT], BF16, name="f1stage")
            nc.sync.dma_start(
                f1stage[0:1, 0:NSLOT],
                f1dram[0:NB, :].rearrange("b q -> () (b q)"))
            nc.gpsimd.partition_broadcast(f1rows[:, :], f1stage[0:1, :])

            # ---- edge phase ----
            for sup in meta["supers"]:
                ntiles = sup["ntiles"]
                gt0 = sup["gt0"]
                ixg = eip.tile([128, NTMAX * 8], I16, name="ixg")
                nc.sync.dma_start(ixg[:, 0:ntiles * 8],
                                  idxg[:, gt0 * 8:(gt0 + ntiles) * 8])
                rr_sb = eip.tile([128, NTMAX], F32, name="rr_sb")
                nc.sync.dma_start(rr_sb[:, 0:ntiles],
                                  rowrel[:, gt0:gt0 + ntiles])

                G = egp.tile([128, NTMAX * 128], BF16, name="G")
                CHUNK = 8          # tiles per dma_gather call (1024-idx cap)
                for g in sup["g_calls"]:
                    r = g["region"]
                    for ct0 in range(0, g["ntiles"], CHUNK):
                        cn = min(CHUNK, g["ntiles"] - ct0)
                        lt0 = g["tile0"] + ct0
                        nc.gpsimd.dma_gather(
                            out_ap=G[:, lt0 * 128:(lt0 + cn) * 128]
                            .rearrange("p (t e) -> p t e", e=OUT),
                            in_ap=tabr[r][:],
                            idxs_ap=ixg[:, lt0 * 8:(lt0 + cn) * 8],
                            num_idxs=cn * 128,
                            num_idxs_reg=cn * 128,
                            elem_size=OUT,
                        )

                F2 = ecp.tile([128, NTMAX], F32, name="F2")
                F1E = ecp.tile([128, NTMAX], F32, name="F1E")
                for blk in sup["blocks"]:
                    b = blk["b"]
                    for (t0, nt) in blk["runs"]:
                        for t in range(t0 - gt0, t0 - gt0 + nt):
                            scr = esc.tile([128, OUT], BF16, name="scr")
                            nc.vector.scalar_tensor_tensor(
                                out=scr[:], in0=G[:, t * 128:(t + 1) * 128],
                                scalar=1.0, in1=arb_sb[:],
                                op0=ALU.mult, op1=ALU.mult,
                                accum_out=F2[:, t:t + 1])
                            scr2 = esc.tile([128, OUT], BF16, name="scr2")
                            nc.vector.scalar_tensor_tensor(
                                out=scr2[:], in0=iota_sb[:],
                                scalar=rr_sb[:, t:t + 1],
                                in1=f1rows[:, b * 128:(b + 1) * 128],
                                op0=ALU.is_equal, op1=ALU.mult,
                                accum_out=F1E[:, t:t + 1])

                # w = exp(0.6 t + 0.4 |t|), t = f1 + f2 (+ bsum via f1rows)
                TT = ecp.tile([128, NTMAX], F32, name="TT")
                nc.vector.tensor_tensor(out=TT[:, 0:ntiles],
                                        in0=F1E[:, 0:ntiles],
                                        in1=F2[:, 0:ntiles], op=ALU.add)
                AA = ecp.tile([128, NTMAX], F32, name="AA")
                nc.scalar.activation(AA[:, 0:ntiles], TT[:, 0:ntiles],
                                     AF.Abs, scale=0.4)
                ZZ = ecp.tile([128, NTMAX], F32, name="ZZ")
                nc.vector.scalar_tensor_tensor(
                    out=ZZ[:, 0:ntiles], in0=TT[:, 0:ntiles], scalar=0.6,
                    in1=AA[:, 0:ntiles], op0=ALU.mult, op1=ALU.add)
                WW = ecp.tile([128, NTMAX], F32, name="WW")
                nc.scalar.activation(WW[:, 0:ntiles], ZZ[:, 0:ntiles], AF.Exp)

                for blk in sup["blocks"]:
                    b = blk["b"]
                    tl = []
                    for (t0, nt) in blk["runs"]:
                        tl.extend(range(t0 - gt0, t0 - gt0 + nt))
                    if not tl:
                        continue
                    ps = epp.tile([128, OUT], F32, name="bps")
                    ps2 = epp.tile([128, 1], F32, name="bps2")
                    for j, t in enumerate(tl):
                        wt = ewp.tile([128, 128], BF16, name="wt")
                        nc.vector.tensor_scalar(
                            out=wt[:], in0=iota_sb[:],
                            scalar1=rr_sb[:, t:t + 1],
                            scalar2=WW[:, t:t + 1],
                            op0=ALU.is_equal, op1=ALU.mult)
                        first = j == 0
                        last = j == len(tl) - 1
                        nc.tensor.matmul(
                            ps[:], lhsT=wt[:],
                            rhs=G[:, t * 128:(t + 1) * 128],
                            start=first, stop=last)
                        nc.tensor.matmul(
                            ps2[:], lhsT=wt[:],
                            rhs=ones_sb[:],
                            start=first, stop=last)
                    sden = eop.tile([128, 1], F32, name="sden")
                    nc.vector.tensor_scalar(out=sden[:],
                                            in0=ps2[:],
                                            scalar1=1e-9, scalar2=None,
                                            op0=ALU.add)
                    rcp = eop.tile([128, 1], F32, name="rcp")
                    nc.vector.reciprocal(rcp[:], sden[:])
                    xx = eop.tile([128, OUT], F32, name="xx")
                    nc.vector.scalar_tensor_tensor(
                        out=xx[:], in0=ps[:], scalar=rcp[:],
                        in1=biasb_sb[:], op0=ALU.mult, op1=ALU.add)
                    ee = eop.tile([128, OUT], F32, name="ee")
                    nc.scalar.activation(ee[:], xx[:], AF.Exp)
                    ov = eop.tile([128, OUT], F32, name="ov")
                    nc.vector.tensor_scalar(out=ov[:], in0=ee[:],
                                            scalar1=-1.0, scalar2=None,
                                            op0=ALU.add)
                    mk = eop.tile([128, OUT], U8, name="mk")
                    nc.vector.tensor_scalar(out=mk[:], in0=xx[:],
                                            scalar1=0.0, scalar2=None,
                                            op0=ALU.is_gt)
                    nc.vector.copy_predicated(ov[:], mk[:], xx[:])
                    nc.sync.dma_start(outp[b * 128:(b + 1) * 128, :], ov[:])

    nc.finalize()
    return nc


def _run(cfg, inputs, trace=False):
    in_maps, assemble = _prep_host(
        cfg,
        np.asarray(inputs["feat"], np.float32),
        np.asarray(inputs["W"], np.float32),
        np.asarray(inputs["a_l"], np.float32),
        np.asarray(inputs["b_l"], np.float32),
        np.asarray(inputs["a_r"], np.float32),
        np.asarray(inputs["b_r"], np.float32),
        np.asarray(inputs["bias"], np.float32),
        np.asarray(inputs["row"]),
        np.asarray(inputs["col"]),
    )
    nc = _build_program(cfg)
    res = run_bass_kernel_spmd(nc, in_maps, list(range(cfg.C)), trace=trace)
    return assemble(res.results), res


def kernel(**inputs):
    feat = np.asarray(inputs["feat"])
    row = np.asarray(inputs["row"])
    cfg = _Cfg(N=feat.shape[0], E=row.shape[0], IN=feat.shape[1],
               OUT=np.asarray(inputs["W"]).shape[1], C=8)
    out, _ = _run(cfg, inputs, trace=False)
    return out


# revision 18
# speedup vs baseline: 1.5680x; 1.5680x over previous
"""GAT attention head (gnn_message_passing) on 8 TRN2 NeuronCores.

v3 design (single gather, pure-block tiles, chunked AllGather):
  - Nodes partitioned across 8 cores (12500 each); per core, dest nodes are
    LPT-balanced into NB=100 blocks of 128 slots (NSLOT=12800).
  - Node phase (region-major, 4 regions of 25 tiles): seq = feat @ W on PE,
    f1 = seq@a_l via fused DVE dot, bf16 seq rows staged per region and
    DMA'd to the AllGather input; after each region completes, a per-region
    AllGather replicates that slot-range of all cores into a region table
    (25600 rows < 32767 so gather indices fit int16).  No table copies.
  - f1 per dest slot is transposed once on PE, round-tripped to a single
    [1, NSLOT] partition-0 row via one SBUF->SBUF DMA, and
    partition-broadcast into f1rows [128, NSLOT] (bf16).
  - Edge phase: edges grouped by (dest block, source region), each
    (block, region) run padded to a multiple of 128 so every 128-edge tile
    is pure (single block, single region).  One dma_gather per
    (super, region) fetches 256B bf16 seq rows.  Per tile:
      F2 (f2 per edge) via fused DVE dot of G with a_r,
      f1e via fused DVE one-hot dot against f1rows (is_equal x mult, accum),
      w = exp(0.6t+0.4|t|+b) as cheap column ops,
      wt = (iota==rowrel)*w in ONE fused DVE op, and PE matmuls
      wt.T@[G] into psum[:, :128] and wt.T@[1] into psum[:, 128:129]
      accumulate numerator and softmax denominator per dest block.
  - Softmax max-subtraction is skipped (logits are O(1); exp safe in f32).

Host side does only index manipulation (partitioning, padding, permutation)
and parameter replication; all floating-point compute on feature data runs
on device.
"""

import math
import sys

import numpy as np

for _p in ("/opt/trn_rl_repo",):
    if _p not in sys.path:
        sys.path.insert(0, _p)

import concourse.bacc as bacc
import concourse.bass as bass
import concourse.mybir as mybir
import concourse.tile as tile
from concourse.bass_utils import run_bass_kernel_spmd

F32 = mybir.dt.float32
BF16 = mybir.dt.bfloat16
I16 = mybir.dt.int16
U8 = mybir.dt.uint8
AF = mybir.ActivationFunctionType
ALU = mybir.AluOpType


class _Cfg:
    def __init__(self, N, E, IN, OUT, C, sup_blocks=8, regions=4):
        assert N % C == 0
        self.N, self.E, self.IN, self.OUT, self.C = N, E, IN, OUT, C
        self.KI = IN // 128
        assert IN == self.KI * 128
        assert OUT == 128, "builder assumes OUT==128"
        self.NPC = N // C                     # nodes per core
        self.REG = regions
        # nodes are partitioned into REG quarters by original id; each
        # quarter gets ceil(QN/128)+1 blocks (one slack block for balance)
        self.QN = math.ceil(self.NPC / regions)   # nodes per quarter
        per_reg = math.ceil(self.QN / 128) + 1
        self.NB = per_reg * regions           # 104 for NPC=12500, REG=4
        self.BPQ = per_reg                    # blocks per quarter
        self.NSLOT = self.NB * 128
        self.RSLOT = self.NSLOT // regions    # slots per region per core
        self.RTB = self.RSLOT // 128          # node tiles per region
        self.RROWS = C * self.RSLOT           # region table rows
        assert self.RROWS <= 32767, "dma_gather int16 index range"
        self.SUP = sup_blocks
        self.supers = []
        b = 0
        while b < self.NB:
            nbl = min(sup_blocks, self.NB - b)
            self.supers.append((b, nbl))
            b += nbl
        self.meta = None


def _prep_host(cfg, feat, W, a_l, b_l, a_r, b_r, bias, row, col):
    C, NPC, NB, NSLOT = cfg.C, cfg.NPC, cfg.NB, cfg.NSLOT
    N, IN, OUT, REG, RSLOT = cfg.N, cfg.IN, cfg.OUT, cfg.REG, cfg.RSLOT

    row = row.astype(np.int64)
    col = col.astype(np.int64)
    core = row // NPC
    QN, BPQ = cfg.QN, cfg.BPQ

    # node -> original-id quarter (this IS the source region, independent of
    # the block assignment below)
    oq = np.minimum((np.arange(N) % NPC) // QN, REG - 1)

    # per-dest in-edge counts split by source region
    vreg = np.zeros((N, REG), np.int64)
    np.add.at(vreg, (row, oq[col]), 1)
    deg = vreg.sum(axis=1)

    # --- region-aware LPT: per (core, quarter), balance per-region loads
    # across the quarter's BPQ blocks (capacity 128 dests each) ------------
    newlocal = np.empty(N, np.int64)
    for c in range(C):
        for q in range(REG):
            n0 = c * NPC + q * QN
            n1 = min(c * NPC + NPC, n0 + QN)
            ids = np.arange(n0, n1)
            order = ids[np.argsort(-deg[n0:n1], kind="stable")]
            loads = np.zeros((BPQ, REG), np.int64)
            counts = np.zeros(BPQ, np.int64)
            vq = vreg[order]
            for i, dest in enumerate(order):
                cand = (loads + vq[i]).max(axis=1)
                cand[counts >= 128] = 1 << 50
                b = int(np.argmin(cand))
                newlocal[dest] = (q * BPQ + b) * 128 + counts[b]
                counts[b] += 1
                loads[b] += vq[i]

    # --- per-edge derived ids --------------------------------------------
    snl = newlocal[col]                       # source slot within its core
    ereg = oq[col]                            # source region (= slot quarter)
    erow = (col // NPC) * RSLOT + (snl - ereg * RSLOT)  # region-local row
    edslot = newlocal[row]                    # dest slot
    eblk = edslot // 128
    epos = (edslot % 128).astype(np.float32)

    # counts per (core, block, region); SPMD => pad to max over cores
    cnts = np.zeros((C, NB, REG), np.int64)
    np.add.at(cnts, (core, eblk, ereg), 1)
    runmax = cnts.max(axis=0)                 # [NB, REG]
    tiles_br = (runmax + 127) // 128          # tiles per (block, region)

    # --- tile layout ------------------------------------------------------
    # G columns are laid out region-major within each super (matching the
    # gather stream); processing is block-major (PSUM locality).
    meta = {"supers": []}
    gtile = 0
    t0_br = np.full((NB, REG), -1, np.int64)  # global first tile of run
    for (b0, nbl) in cfg.supers:
        sup = {"b0": b0, "nb": nbl, "gt0": gtile, "g_calls": [], "blocks": []}
        scol = 0
        for r in range(REG):
            nt_r = int(tiles_br[b0:b0 + nbl, r].sum())
            if nt_r:
                sup["g_calls"].append(
                    {"region": r, "tile0": scol, "ntiles": nt_r})
            for bi in range(nbl):
                t0_br[b0 + bi, r] = gtile + scol
                scol += int(tiles_br[b0 + bi, r])
        sup["ntiles"] = scol
        for bi in range(nbl):
            b = b0 + bi
            runs = [(int(t0_br[b, r]), int(tiles_br[b, r]))
                    for r in range(REG) if tiles_br[b, r] > 0]
            sup["blocks"].append({"b": b, "runs": runs})
        gtile += scol
        meta["supers"].append(sup)
    NTILES = gtile
    meta["NTILES"] = NTILES

    # --- per-core index arrays -------------------------------------------
    idxg = np.zeros((C, 128, NTILES * 8), np.int16)
    rowrel = np.full((C, 128, NTILES), -1.0, np.float32)

    # slot of each edge within its (core, block, region) run
    okey = (core * NB + eblk) * REG + ereg
    oorder = np.argsort(okey, kind="stable")
    ks = okey[oorder]
    starts = np.searchsorted(ks, np.arange(C * NB * REG))
    slot_in_run = np.empty(cfg.E, np.int64)
    slot_in_run[oorder] = np.arange(cfg.E) - starts[ks]

    gt0_e = t0_br[eblk, ereg]                 # first global tile of the run
    ek = gt0_e * 128 + slot_in_run            # global slot id
    etile = ek // 128
    epart = ek % 128

    for c in range(C):
        m = core == c
        rowrel[c, epart[m], etile[m]] = epos[m]
        # wrapped idx layout: slot k -> [k%16, (k//16)], replicated to the
        # 8 groups of 16 partitions
        kk = ek[m]
        idxg[c, kk % 16, (kk // 16)] = erow[m].astype(np.int16)
    for g in range(1, 8):
        idxg[:, g * 16:(g + 1) * 16, :] = idxg[:, 0:16, :]

    # --- parameters -------------------------------------------------------
    inv = np.empty((C, NSLOT), np.int64)
    have = np.zeros((C, NSLOT), bool)
    for c in range(C):
        nl = newlocal[c * NPC:(c + 1) * NPC]
        inv[c, nl] = np.arange(NPC)
        have[c, nl] = True
    featT = np.zeros((C, IN, NSLOT), np.float32)
    for c in range(C):
        idx = inv[c][have[c]]
        featT[c][:, have[c]] = feat[c * NPC + idx].T
    wks = [np.ascontiguousarray(W[k * 128:(k + 1) * 128]).astype(np.float32)
           for k in range(cfg.KI)]
    alb = np.tile(np.asarray(a_l, np.float32)[None, :], (128, 1))
    arb = np.tile(np.asarray(a_r, np.float32).astype(np.float32)[None, :],
                  (128, 1)).astype(np.float32)
    arbh = arb.astype(np.float32)
    biasb = np.tile(np.asarray(bias, np.float32)[None, :], (128, 1))
    bsum = float(np.asarray(b_l, np.float64) + np.asarray(b_r, np.float64))
    bs_col = np.full((128, 1), bsum, np.float32)
    iota = np.tile(np.arange(128, dtype=np.float32)[None, :], (128, 1))
    ident = np.eye(128, dtype=np.float32)

    import ml_dtypes
    iotab = iota.astype(ml_dtypes.bfloat16)
    arb_bf = arbh.astype(ml_dtypes.bfloat16)

    in_maps = []
    for c in range(C):
        m = {
            "featT": featT[c], "alb": alb, "arbh": arb_bf, "biasb": biasb,
            "bs": bs_col, "iotab": iotab, "ident": ident,
            "idxg": idxg[c], "rowrel": rowrel[c],
        }
        for k in range(cfg.KI):
            m[f"wk{k}"] = wks[k]
        in_maps.append(m)

    cfg.meta = meta

    def assemble(outs):
        full = np.empty((N, OUT), np.float32)
        for c in range(C):
            o = outs[c]["out"]
            nlc = newlocal[c * NPC:(c + 1) * NPC]
            full[c * NPC:(c + 1) * NPC] = o[nlc]
        return full

    return in_maps, assemble


def _build_program(cfg):
    C, IN, OUT = cfg.C, cfg.IN, cfg.OUT
    NB, NSLOT, KI, REG = cfg.NB, cfg.NSLOT, cfg.KI, cfg.REG
    RSLOT, RTB, RROWS = cfg.RSLOT, cfg.RTB, cfg.RROWS
    meta = cfg.meta
    NTILES = meta["NTILES"]
    NTMAX = max(s["ntiles"] for s in meta["supers"])
    FTC = RTB // 2 if RTB % 2 == 0 else RTB     # node tiles per featT chunk
    FCH = FTC * 128                             # featT load chunk columns

    nc = bacc.Bacc(None, num_swdge_queues=4)
    featT = nc.declare_dram_parameter("featT", [IN, NSLOT], F32, isOutput=False)
    wk = [nc.declare_dram_parameter(f"wk{k}", [128, OUT], F32, isOutput=False)
          for k in range(KI)]
    alb = nc.declare_dram_parameter("alb", [128, OUT], F32, isOutput=False)
    arbh = nc.declare_dram_parameter("arbh", [128, OUT], BF16, isOutput=False)
    biasb = nc.declare_dram_parameter("biasb", [128, OUT], F32, isOutput=False)
    bs = nc.declare_dram_parameter("bs", [128, 1], F32, isOutput=False)
    iotab = nc.declare_dram_parameter("iotab", [128, 128], BF16, isOutput=False)
    ident = nc.declare_dram_parameter("ident", [128, 128], F32, isOutput=False)
    idxg = nc.declare_dram_parameter("idxg", [128, NTILES * 8], I16,
                                     isOutput=False)
    rowrel = nc.declare_dram_parameter("rowrel", [128, NTILES], F32,
                                       isOutput=False)
    outp = nc.declare_dram_parameter("out", [NSLOT, OUT], F32, isOutput=True)

    with tile.TileContext(nc) as tc:
        with (
            tc.tile_pool(name="dram", bufs=1, space="DRAM") as dram,
            tc.tile_pool(name="consts", bufs=1) as cp,
            tc.tile_pool(name="nfeat", bufs=2) as nfp,
            tc.tile_pool(name="naug", bufs=2) as nap,
            tc.tile_pool(name="nscr", bufs=2) as nsp,
            tc.tile_pool(name="npsum", bufs=2, space="PSUM") as npp,
            tc.tile_pool(name="eidx", bufs=2) as eip,
            tc.tile_pool(name="egath", bufs=2) as egp,
            tc.tile_pool(name="ecol", bufs=2) as ecp,
            tc.tile_pool(name="escr", bufs=2) as esc,
            tc.tile_pool(name="ewt", bufs=4) as ewp,
            tc.tile_pool(name="epsum", bufs=2, space="PSUM") as epp,
            tc.tile_pool(name="eout", bufs=3) as eop,
        ):
            agin = [dram.tile([RSLOT, OUT], BF16, name=f"agin{r}")
                    for r in range(REG)]
            tabr = [dram.tile([RROWS, OUT], BF16, name=f"tabr{r}",
                              addr_space="Shared") for r in range(REG)]
            f1dram = dram.tile([128, 128], BF16)

            # ---- constants ----
            wk_sb = []
            for k in range(KI):
                w_t = cp.tile([128, OUT], F32, name=f"wksb{k}")
                nc.sync.dma_start(w_t[:], wk[k][:])
                wk_sb.append(w_t)
            alb_sb = cp.tile([128, OUT], F32)
            nc.sync.dma_start(alb_sb[:], alb[:])
            arb_sb = cp.tile([128, OUT], BF16)
            nc.sync.dma_start(arb_sb[:], arbh[:])
            biasb_sb = cp.tile([128, OUT], F32)
            nc.sync.dma_start(biasb_sb[:], biasb[:])
            bs_sb = cp.tile([128, 1], F32)
            nc.sync.dma_start(bs_sb[:], bs[:])
            iota_sb = cp.tile([128, 128], BF16)
            nc.sync.dma_start(iota_sb[:], iotab[:])
            ident_sb = cp.tile([128, 128], F32)
            nc.sync.dma_start(ident_sb[:], ident[:])
            ones_sb = cp.tile([128, 1], BF16)
            nc.vector.memset(ones_sb[:], 1.0)
            f1acc = cp.tile([128, 128], F32)
            nc.vector.memset(f1acc[:], 0.0)
            f1rows = cp.tile([128, NSLOT], BF16)

            # ---- node phase (region-major) ----
            for r in range(REG):
                fts = {}
                for k in range(KI):
                    for h in range(RTB // FTC):
                        ft = nfp.tile([128, FCH], F32, name=f"ft{k}{h}")
                        c0 = r * RSLOT + h * FCH
                        nc.sync.dma_start(
                            ft[:], featT[k * 128:(k + 1) * 128, c0:c0 + FCH])
                        fts[(k, h)] = ft
                aug = nap.tile([128, RSLOT], BF16, name="aug")
                for ntl in range(RTB):
                    nt = r * RTB + ntl
                    ps = npp.tile([128, OUT], F32)
                    for k in range(KI):
                        h, off = divmod(ntl * 128, FCH)
                        ft = fts[(k, h)]
                        nc.tensor.matmul(ps[:], lhsT=ft[:, off:off + 128],
                                         rhs=wk_sb[k][:],
                                         start=(k == 0), stop=(k == KI - 1))
                    nc.vector.tensor_copy(aug[:, ntl * 128:(ntl + 1) * 128],
                                          ps[:])
                    scr1 = nsp.tile([128, OUT], F32, name="scr1")
                    nc.vector.scalar_tensor_tensor(
                        out=scr1[:], in0=ps[:], scalar=1.0, in1=alb_sb[:],
                        op0=ALU.mult, op1=ALU.mult,
                        accum_out=f1acc[:, nt:nt + 1])
                nc.sync.dma_start(
                    agin[r][:, :].rearrange("(t p) o -> p t o", p=128),
                    aug[:].rearrange("p (t o) -> p t o", o=OUT))
                # per-region AllGather into the region table
                nc.gpsimd.collective_compute(
                    "AllGather", ALU.bypass,
                    replica_groups=[list(range(C))],
                    ins=[agin[r].opt()],
                    outs=[tabr[r].opt()],
                )

            # ---- f1rows: f1acc [128(pos), NB] -> broadcast rows ----
            f1b = nsp.tile([128, 128], F32, name="f1b")
            nc.vector.tensor_scalar(out=f1b[:], in0=f1acc[:],
                                    scalar1=bs_sb[:], scalar2=None,
                                    op0=ALU.add)
            f1ps = npp.tile([128, 128], F32)
            nc.tensor.transpose(f1ps[:], f1b[:], ident_sb[:])
            f1sbT = nsp.tile([128, 128], BF16, name="f1sbT")
            nc.vector.tensor_copy(f1sbT[:], f1ps[:])
            nc.sync.dma_start(f1dram[:], f1sbT[:])
            f1stage = cp.tile([1, NSLOT], BF16, name="f1stage")
            nc.sync.dma_start(
                f1stage[0:1, 0:NSLOT],
                f1dram[0:NB, :].rearrange("b q -> () (b q)"))
            nc.gpsimd.partition_broadcast(f1rows[:, :], f1stage[0:1, :])

            # ---- edge phase ----
            self_qn = [0]
            for sup in meta["supers"]:
                ntiles = sup["ntiles"]
                gt0 = sup["gt0"]
                ixg = eip.tile([128, NTMAX * 8], I16, name="ixg")
                nc.sync.dma_start(ixg[:, 0:ntiles * 8],
                                  idxg[:, gt0 * 8:(gt0 + ntiles) * 8])
                rr_sb = eip.tile([128, NTMAX], F32, name="rr_sb")
                nc.sync.dma_start(rr_sb[:, 0:ntiles],
                                  rowrel[:, gt0:gt0 + ntiles])

                G = egp.tile([128, NTMAX * 128], BF16, name="G")
                CHUNK = 8          # tiles per dma_gather call (1024-idx cap)
                for g in sup["g_calls"]:
                    r = g["region"]
                    for ct0 in range(0, g["ntiles"], CHUNK):
                        cn = min(CHUNK, g["ntiles"] - ct0)
                        lt0 = g["tile0"] + ct0
                        nc.gpsimd.dma_gather(
                            out_ap=G[:, lt0 * 128:(lt0 + cn) * 128]
                            .rearrange("p (t e) -> p t e", e=OUT),
                            in_ap=tabr[r][:],
                            idxs_ap=ixg[:, lt0 * 8:(lt0 + cn) * 8],
                            num_idxs=cn * 128,
                            num_idxs_reg=cn * 128,
                            elem_size=OUT,
                            queue_num=self_qn[0] % 4,
                        )
                        self_qn[0] += 1

                F2 = ecp.tile([128, NTMAX], F32, name="F2")
                F1E = ecp.tile([128, NTMAX], F32, name="F1E")
                for blk in sup["blocks"]:
                    b = blk["b"]
                    for (t0, nt) in blk["runs"]:
                        for t in range(t0 - gt0, t0 - gt0 + nt):
                            scr = esc.tile([128, OUT], BF16, name="scr")
                            nc.vector.scalar_tensor_tensor(
                                out=scr[:], in0=G[:, t * 128:(t + 1) * 128],
                                scalar=1.0, in1=arb_sb[:],
                                op0=ALU.mult, op1=ALU.mult,
                                accum_out=F2[:, t:t + 1])
                            scr2 = esc.tile([128, OUT], BF16, name="scr2")
                            nc.vector.scalar_tensor_tensor(
                                out=scr2[:], in0=iota_sb[:],
                                scalar=rr_sb[:, t:t + 1],
                                in1=f1rows[:, b * 128:(b + 1) * 128],
                                op0=ALU.is_equal, op1=ALU.mult,
                                accum_out=F1E[:, t:t + 1])

                # w = exp(0.6 t + 0.4 |t|), t = f1 + f2 (+ bsum via f1rows)
                TT = ecp.tile([128, NTMAX], F32, name="TT")
                nc.vector.tensor_tensor(out=TT[:, 0:ntiles],
                                        in0=F1E[:, 0:ntiles],
                                        in1=F2[:, 0:ntiles], op=ALU.add)
                AA = ecp.tile([128, NTMAX], F32, name="AA")
                nc.scalar.activation(AA[:, 0:ntiles], TT[:, 0:ntiles],
                                     AF.Abs, scale=0.4)
                ZZ = ecp.tile([128, NTMAX], F32, name="ZZ")
                nc.vector.scalar_tensor_tensor(
                    out=ZZ[:, 0:ntiles], in0=TT[:, 0:ntiles], scalar=0.6,
                    in1=AA[:, 0:ntiles], op0=ALU.mult, op1=ALU.add)
                WW = ecp.tile([128, NTMAX], F32, name="WW")
                nc.scalar.activation(WW[:, 0:ntiles], ZZ[:, 0:ntiles], AF.Exp)

                for blk in sup["blocks"]:
                    b = blk["b"]
                    tl = []
                    for (t0, nt) in blk["runs"]:
                        tl.extend(range(t0 - gt0, t0 - gt0 + nt))
                    if not tl:
                        continue
                    ps = epp.tile([128, OUT], F32, name="bps")
                    ps2 = epp.tile([128, 1], F32, name="bps2")
                    for j, t in enumerate(tl):
                        wt = ewp.tile([128, 128], BF16, name="wt")
                        nc.vector.tensor_scalar(
                            out=wt[:], in0=iota_sb[:],
                            scalar1=rr_sb[:, t:t + 1],
                            scalar2=WW[:, t:t + 1],
                            op0=ALU.is_equal, op1=ALU.mult)
                        first = j == 0
                        last = j == len(tl) - 1
                        nc.tensor.matmul(
                            ps[:], lhsT=wt[:],
                            rhs=G[:, t * 128:(t + 1) * 128],
                            start=first, stop=last)
                        nc.tensor.matmul(
                            ps2[:], lhsT=wt[:],
                            rhs=ones_sb[:],
                            start=first, stop=last)
                    sden = eop.tile([128, 1], F32, name="sden")
                    nc.vector.tensor_scalar(out=sden[:],
                                            in0=ps2[:],
                                            scalar1=1e-9, scalar2=None,
                                            op0=ALU.add)
                    rcp = eop.tile([128, 1], F32, name="rcp")
                    nc.vector.reciprocal(rcp[:], sden[:])
                    xx = eop.tile([128, OUT], F32, name="xx")
                    nc.vector.scalar_tensor_tensor(
                        out=xx[:], in0=ps[:], scalar=rcp[:],
                        in1=biasb_sb[:], op0=ALU.mult, op1=ALU.add)
                    ee = eop.tile([128, OUT], F32, name="ee")
                    nc.scalar.activation(ee[:], xx[:], AF.Exp)
                    ov = eop.tile([128, OUT], F32, name="ov")
                    nc.vector.tensor_scalar(out=ov[:], in0=ee[:],
                                            scalar1=-1.0, scalar2=None,
                                            op0=ALU.add)
                    mk = eop.tile([128, OUT], U8, name="mk")
                    nc.vector.tensor_scalar(out=mk[:], in0=xx[:],
                                            scalar1=0.0, scalar2=None,
                                            op0=ALU.is_gt)
                    nc.vector.copy_predicated(ov[:], mk[:], xx[:])
                    nc.sync.dma_start(outp[b * 128:(b + 1) * 128, :], ov[:])

    nc.finalize()
    return nc


def _run(cfg, inputs, trace=False):
    in_maps, assemble = _prep_host(
        cfg,
        np.asarray(inputs["feat"], np.float32),
        np.asarray(inputs["W"], np.float32),
        np.asarray(inputs["a_l"], np.float32),
        np.asarray(inputs["b_l"], np.float32),
        np.asarray(inputs["a_r"], np.float32),
        np.asarray(inputs["b_r"], np.float32),
        np.asarray(inputs["bias"], np.float32),
        np.asarray(inputs["row"]),
        np.asarray(inputs["col"]),
    )
    nc = _build_program(cfg)
    res = run_bass_kernel_spmd(nc, in_maps, list(range(cfg.C)), trace=trace)
    return assemble(res.results), res


def kernel(**inputs):
    feat = np.asarray(inputs["feat"])
    row = np.asarray(inputs["row"])
    cfg = _Cfg(N=feat.shape[0], E=row.shape[0], IN=feat.shape[1],
               OUT=np.asarray(inputs["W"]).shape[1], C=8)
    out, _ = _run(cfg, inputs, trace=False)
    return out
